# revision 22
# baseline (speedup 1.0000x reference)
"""Trainium2 Bass kernel for nn_CombinedLoss (MSE + pairwise adaptive-boundary
ranking loss over all pairs i<j of B=8192 elements).

Strategy (v2)
-------------
Sort (pred, target) by target on the host (the loss is permutation
invariant); then for sorted i<j:

    pair_loss[i,j] = relu(P(e) - (p_j - p_i)),   e = t_j - t_i >= 0

with P(e) = BETA*e/(1+GAMMA*e) replaced by its degree-D Taylor polynomial
(D=3, truncation bias ~3e-6 relative).  Expanding P in powers of t_j makes
m[i,j] a rank-(D+2) product computed by the TensorEngine:

    m = lhsT.T @ V,  V = [1, t_j, .., t_j^D, p_j]   (KDIM x B, bf16)
    lhsT[:,i] = [A_0(t_i)+p_i, A_1(t_i), .., A_D(t_i), -1]

Everything runs in single bf16 (no hi/lo split): the rel-err budget is 2e-2
and measured error of this scheme is ~3e-6.

Sharding: 64 row-blocks of 128 rows; core c takes row-blocks {8s+c}.  Slot s
needs column blocks b = s..7, so every core runs an identical SPMD schedule
over 36 (s,b) chunks of [128 x 1024].  No mask tensor: the lower-triangle
garbage (j <= i) inside the 8 diagonal chunks is reproduced EXACTLY on the
host (same bf16 inputs, f32 accumulation) and subtracted from the final
scalar, as are exact-tie pairs (reference gives them 0 via sign(0)).

PE: K = KDIM = 5 <= 32, so matmuls are 4-way row-tiled (tile_position
(32g, 0)) with A and V replicated at partition bases {0,32,64,96}; four
chunks are in flight concurrently, which keeps the (possibly cold-clocked)
PE well ahead of the reducers.

Reduce: the bottleneck.  PSUM-source ops run at 1 elem/cycle/lane, so the 36
chunk reductions are split between ScalarE (ACT Relu + accum_out) and
VectorE (DVE max0 + add-reduce), each chunk one fused instruction into a
distinct accumulator column.  Host sums the accumulator tiles.
"""

import numpy as np
from math import comb

B = 8192
NCORES = 8
NSLOTS = 8
D = 3           # polynomial degree (truncation bias ~3e-6 relative)
KDIM = D + 2    # contraction rows: ones, t^1..t^D, p
BETA = 0.3
GAMMA = 0.1
MSE_WEIGHT = 1.0
RANK_WEIGHT = 1.0

# chunk list: (slot s, column block b) with b >= s; ordered by b so compute
# follows DMA arrival of V column blocks.  The diagonal chunk (s, s) is
# processed as a HALF chunk (cols [512, 1024) of its block): its left half
# is garbage (j <= i) for cores 4-7 and is reconstructed exactly on the
# host for cores 0-3.  This trims 11% of the reduce work on every core.
CHUNKS = [(s, b) for b in range(NSLOTS) for s in range(b + 1)]
NCHUNKS = len(CHUNKS)          # 36
DIAG = {i for i, (s, b) in enumerate(CHUNKS) if s == b}
# engine split: measured per-chunk cost is ~1.19us on ACT (ACTIVATE +
# READ_ACCUMULATOR) and ~1.22us on DVE -> 18/18, diagonals split 4/4.
ACT_IDX = [i for i in range(NCHUNKS) if i % 2 == 0]
NACT = len(ACT_IDX)

_CACHE: dict = {}


def _poly_coeffs():
    # P(a) = sum_{n=1..D} c_n a^n,  c_n = BETA * (-GAMMA)^(n-1)
    return np.array([BETA * (-GAMMA) ** (n - 1) for n in range(1, D + 1)],
                    dtype=np.float64)


def _build_program():
    import concourse.bass as bass
    import concourse.bacc as bacc
    import concourse.tile as tile
    import concourse.mybir as mybir

    f32 = mybir.dt.float32
    bf16 = mybir.dt.bfloat16
    Alu = mybir.AluOpType
    Act = mybir.ActivationFunctionType

    nc = bacc.Bacc("TRN2", target_bir_lowering=False, debug=False,
                   num_devices=NCORES)

    A_d = nc.dram_tensor("AR", [128, 1024], bf16, kind="ExternalInput")
    V_d = nc.dram_tensor("VR", [128, B], bf16, kind="ExternalInput")
    TP_d = nc.dram_tensor("TP", [128, 128], f32, kind="ExternalInput")
    Q_d = nc.dram_tensor("Q", [128, NCHUNKS + 1], f32, kind="ExternalOutput")

    with tile.TileContext(nc) as tc:
        with (
            tc.tile_pool(name="const", bufs=1) as cp,
            tc.tile_pool(name="pa", bufs=2, space="PSUM") as pap,
            tc.tile_pool(name="pv", bufs=2, space="PSUM") as pvp,
        ):
            A_sb = cp.tile([128, 1024], bf16)
            V_sb = cp.tile([128, B], bf16)
            TP_sb = cp.tile([128, 128], f32)
            q = cp.tile([128, NCHUNKS + 1], f32)

            # DMA plan.  V/A are host-replicated at the 4 PE row-group
            # partition bases.  Critical pieces (A slots 0-1, V block-0
            # right half, V block 1) go full-partition on sync in column
            # pieces (the dep tracker is column-granular); the bulk (blocks
            # 2-7) is read per row-group straight from the replicated blob
            # (4 x 60KB on gpsimd).  V cols [0:512) are used by nothing
            # (block 0 only has its trimmed diagonal chunk).  The Scalar
            # engine issues no DMAs so its ACT_TABLE_LOAD runs at t=0.
            nc.sync.dma_start(A_sb[:, 0:256], A_d[:, 0:256])
            nc.sync.dma_start(V_sb[:, 512:1024], V_d[:, 512:1024])
            nc.sync.dma_start(V_sb[:, 1024:2048], V_d[:, 1024:2048])
            nc.sync.dma_start(A_sb[:, 256:1024], A_d[:, 256:1024])
            for g in range(4):
                nc.gpsimd.dma_start(
                    V_sb[32 * g:32 * g + KDIM, 2048:8192],
                    V_d[32 * g:32 * g + KDIM, 2048:8192])
            nc.sync.dma_start(TP_sb[:], TP_d[:])

            # Emit matmuls in rounds with interleaved halves so consecutive
            # MMs target different PE row-groups (4-way overlap).  Early
            # rounds are small so the first chunks only need the V blocks
            # that have already arrived (no PE FIFO head-of-line stalls).
            rounds = []
            pos = 0
            for size in (1, 2, 3):
                rounds.append(list(range(pos, pos + size)))
                pos += size
            while pos < NCHUNKS:
                rounds.append(list(range(pos, min(pos + 4, NCHUNKS))))
                pos += 4
            ka = 0
            kv = NACT + 1
            act_set = set(ACT_IDX)
            for ridx, grp in enumerate(rounds):
                pss = {}
                for i in grp:
                    on_act = i in act_set
                    pool = pap if on_act else pvp
                    w = 512 if i in DIAG else 1024
                    pss[i] = pool.tile([128, w], f32,
                                       tag="pa" if on_act else "pv",
                                       name=f"ps{i}")
                # DVE-destined chunks first within each wave: DVE frees its
                # tiles sooner, so the PE never stalls ahead of DVE work.
                order = sorted(grp, key=lambda i: (i in act_set, i))
                for h in range(2):
                    for i in order:
                        if h == 1 and i in DIAG:
                            continue
                        s, b = CHUNKS[i]
                        g = 32 * (i % 4)
                        # diagonal chunks cover only the right half of
                        # their block
                        c0 = 1024 * b + 512 * (1 if i in DIAG else h)
                        nc.tensor.matmul(
                            pss[i][:, 512 * h:512 * (h + 1)],
                            A_sb[g:g + KDIM, 128 * s:128 * (s + 1)],
                            V_sb[g:g + KDIM, c0:c0 + 512],
                            start=True, stop=True,
                            tile_position=(g, 0),
                        )
                for i in order:
                    ps = pss[i]
                    if i in act_set:
                        nc.scalar.activation(ps[:], ps[:], Act.Relu,
                                             accum_out=q[:, ka:ka + 1])
                        ka += 1
                    else:
                        nc.vector.tensor_scalar(
                            ps[:], ps[:], 0.0, None, op0=Alu.max,
                            op1=Alu.add, accum_out=q[:, kv:kv + 1],
                        )
                        kv += 1
            assert ka == NACT and kv == NCHUNKS + 1

            # MSE, off the reducers' critical path: subtract on GpSimd (idle
            # engine; waits on the late TP DMA without blocking DVE), square
            # + accumulate on ACT at the very end.
            d_sb = cp.tile([128, 64], f32)
            nc.gpsimd.tensor_sub(d_sb[:], TP_sb[:, 64:128], TP_sb[:, 0:64])
            msq = cp.tile([128, 64], f32)
            nc.scalar.activation(msq[:], d_sb[:], Act.Square,
                                 accum_out=q[:, NACT:NACT + 1])

            nc.sync.dma_start(Q_d[:], q[:])

    nc.compile()
    return nc


def _host_inputs(pred: np.ndarray, target: np.ndarray):
    """Sort by target; build bf16 A (replicated per core) and V (replicated,
    shared); compute the exact correction: garbage lower-triangle content of
    the 8 diagonal chunks per core, plus exact-tie pairs."""
    import ml_dtypes

    order = np.argsort(target, kind="stable")
    ts32 = target[order]
    ps32 = pred[order]
    ts = ts32.astype(np.float64)
    ps = ps32.astype(np.float64)

    c = _poly_coeffs()
    V = np.empty((KDIM, B), dtype=np.float64)
    V[0] = 1.0
    for k in range(1, D + 1):
        V[k] = ts ** k
    V[KDIM - 1] = ps

    # A_k(t_i) = sum_{n >= max(k,1)} c_n * C(n,k) * (-t_i)^(n-k)
    A = np.zeros((KDIM, B), dtype=np.float64)
    for k in range(0, D + 1):
        for n in range(max(k, 1), D + 1):
            A[k] += c[n - 1] * comb(n, k) * (-ts) ** (n - k)
    A[0] += ps          # fold +p_i into the constant row
    A[KDIM - 1] = -1.0

    Vq = V.astype(ml_dtypes.bfloat16)
    Aq = A.astype(ml_dtypes.bfloat16)
    Vf = Vq.astype(np.float32)
    Af = Aq.astype(np.float32)

    VR = np.zeros((128, B), dtype=ml_dtypes.bfloat16)
    for g in range(4):
        VR[32 * g:32 * g + KDIM, :] = Vq

    TP = np.zeros((128, 128), dtype=np.float32)
    TP[:, 0:64] = ts32.reshape(128, 64)
    TP[:, 64:128] = ps32.reshape(128, 64)

    jloc = np.arange(1024)[None, :]
    prow = np.arange(128)[:, None]
    in_maps = []
    garbage = np.float64(0.0)  # summed over all cores
    for core in range(NCORES):
        rows = np.concatenate(
            [128 * (8 * s + core) + np.arange(128) for s in range(NSLOTS)])
        Acore = Aq[:, rows]                       # [KDIM, 1024]
        AR = np.zeros((128, 1024), dtype=ml_dtypes.bfloat16)
        for g in range(4):
            AR[32 * g:32 * g + KDIM, :] = Acore
        in_maps.append({"AR": AR, "VR": VR, "TP": TP})

        # Diagonal chunk (s, b=s): the device processes only jloc in
        # [512, 1024).  Net correction = included-garbage (jloc >= 512 and
        # j <= i) minus missing-real (jloc < 512 and j > i).
        Acf = Acore.astype(np.float32)
        for s in range(NSLOTS):
            m = (Acf[:, 128 * s:128 * (s + 1)].T
                 @ Vf[:, 1024 * s:1024 * (s + 1)])
            r = np.maximum(m, np.float32(0.0)).astype(np.float64)
            proc = jloc >= 512
            tri = jloc > (128 * core + prow)
            garbage += r[proc & ~tri].sum()
            garbage -= r[~proc & tri].sum()

    # exact ties (t_i == t_j in fp32, i<j sorted): device adds
    # relu(Af(:,i).Vf(:,j)); reference wants 0.  Subtract device value.
    ties = np.float64(0.0)
    uq, inv, cnt = np.unique(ts32, return_inverse=True, return_counts=True)
    for gidx in np.nonzero(cnt > 1)[0]:
        idx = np.nonzero(inv == gidx)[0]
        for a in range(len(idx)):
            for bb in range(a + 1, len(idx)):
                i, j = idx[a], idx[bb]
                mv = np.float32(Af[:, i] @ Vf[:, j])
                ties += max(float(mv), 0.0)

    return in_maps, garbage + ties


def kernel(pred: np.ndarray, target: np.ndarray):
    from concourse.bass_utils import run_bass_kernel_spmd

    pred = np.ascontiguousarray(np.asarray(pred, dtype=np.float32))
    target = np.ascontiguousarray(np.asarray(target, dtype=np.float32))
    assert pred.shape == (B,) and target.shape == (B,)

    if "nc" not in _CACHE:
        _CACHE["nc"] = _build_program()
    nc = _CACHE["nc"]

    in_maps, correction = _host_inputs(pred, target)
    res = run_bass_kernel_spmd(nc, in_maps, list(range(NCORES)))
    _CACHE["last_results"] = res

    total = np.float64(0.0)
    for core in range(NCORES):
        qq = res.results[core]["Q"].astype(np.float64)
        total += qq[:, :NACT].sum() + qq[:, NACT + 1:].sum()
    K = B * (B - 1) // 2
    rank = (total - correction) / K
    mse = res.results[0]["Q"][:, NACT].astype(np.float64).sum() / B
    combined = MSE_WEIGHT * mse + RANK_WEIGHT * rank
    return (
        np.float32(combined),
        np.float32(mse),
        np.float32(rank),
    )


# revision 23
# speedup vs baseline: 1.0995x; 1.0995x over previous
"""Trainium2 Bass kernel for nn_CombinedLoss (MSE + pairwise adaptive-boundary
ranking loss over all pairs i<j of B=8192 elements).

Strategy (v2)
-------------
Sort (pred, target) by target on the host (the loss is permutation
invariant); then for sorted i<j:

    pair_loss[i,j] = relu(P(e) - (p_j - p_i)),   e = t_j - t_i >= 0

with P(e) = BETA*e/(1+GAMMA*e) replaced by its degree-D Taylor polynomial
(D=3, truncation bias ~3e-6 relative).  Expanding P in powers of t_j makes
m[i,j] a rank-(D+2) product computed by the TensorEngine:

    m = lhsT.T @ V,  V = [1, t_j, .., t_j^D, p_j]   (KDIM x B, bf16)
    lhsT[:,i] = [A_0(t_i)+p_i, A_1(t_i), .., A_D(t_i), -1]

Everything runs in single bf16 (no hi/lo split): the rel-err budget is 2e-2
and measured error of this scheme is ~3e-6.

Sharding: 64 row-blocks of 128 rows; core c takes row-blocks {8s+c}.  Slot s
needs column blocks b = s..7, so every core runs an identical SPMD schedule
over 36 (s,b) chunks of [128 x 1024].  No mask tensor: the lower-triangle
garbage (j <= i) inside the 8 diagonal chunks is reproduced EXACTLY on the
host (same bf16 inputs, f32 accumulation) and subtracted from the final
scalar, as are exact-tie pairs (reference gives them 0 via sign(0)).

PE: K = KDIM = 5 <= 32, so matmuls are 4-way row-tiled (tile_position
(32g, 0)) with A and V replicated at partition bases {0,32,64,96}; four
chunks are in flight concurrently, which keeps the (possibly cold-clocked)
PE well ahead of the reducers.

Reduce: the bottleneck.  PSUM-source ops run at 1 elem/cycle/lane, so the 36
chunk reductions are split between ScalarE (ACT Relu + accum_out) and
VectorE (DVE max0 + add-reduce), each chunk one fused instruction into a
distinct accumulator column.  Host sums the accumulator tiles.
"""

import numpy as np
from math import comb

B = 8192
NCORES = 8
NSLOTS = 8
D = 3           # polynomial degree (truncation bias ~3e-6 relative)
KDIM = D + 2    # contraction rows: ones, t^1..t^D, p
BETA = 0.3
GAMMA = 0.1
MSE_WEIGHT = 1.0
RANK_WEIGHT = 1.0

# chunk list: (slot s, column block b) with b >= s; ordered by b so compute
# follows DMA arrival of V column blocks.  The diagonal chunk (s, s) is
# processed as a HALF chunk (cols [512, 1024) of its block): its left half
# is garbage (j <= i) for cores 4-7 and is reconstructed exactly on the
# host for cores 0-3.  This trims 11% of the reduce work on every core.
CHUNKS = [(s, b) for b in range(NSLOTS) for s in range(b + 1)]
NCHUNKS = len(CHUNKS)          # 36
DIAG = {i for i, (s, b) in enumerate(CHUNKS) if s == b}
# engine split: measured per-chunk cost is ~1.19us on ACT (ACTIVATE +
# READ_ACCUMULATOR) and ~1.22us on DVE -> 18/18, diagonals split 4/4.
ACT_IDX = [i for i in range(NCHUNKS) if i % 2 == 0]
NACT = len(ACT_IDX)

_CACHE: dict = {}


def _poly_coeffs():
    # P(a) = sum_{n=1..D} c_n a^n,  c_n = BETA * (-GAMMA)^(n-1)
    return np.array([BETA * (-GAMMA) ** (n - 1) for n in range(1, D + 1)],
                    dtype=np.float64)


def _build_program():
    import concourse.bass as bass
    import concourse.bacc as bacc
    import concourse.tile as tile
    import concourse.mybir as mybir

    f32 = mybir.dt.float32
    bf16 = mybir.dt.bfloat16
    Alu = mybir.AluOpType
    Act = mybir.ActivationFunctionType

    nc = bacc.Bacc("TRN2", target_bir_lowering=False, debug=False,
                   num_devices=NCORES)

    A_d = nc.dram_tensor("AR", [128, 1024], bf16, kind="ExternalInput")
    V_d = nc.dram_tensor("VR", [128, B], bf16, kind="ExternalInput")
    TP_d = nc.dram_tensor("TP", [128, 128], f32, kind="ExternalInput")
    Q_d = nc.dram_tensor("Q", [128, NCHUNKS + 1], f32, kind="ExternalOutput")

    with tile.TileContext(nc) as tc:
        with (
            tc.tile_pool(name="const", bufs=1) as cp,
            tc.tile_pool(name="pa", bufs=2, space="PSUM") as pap,
            tc.tile_pool(name="pv", bufs=2, space="PSUM") as pvp,
        ):
            A_sb = cp.tile([128, 1024], bf16)
            V_sb = cp.tile([128, B], bf16)
            TP_sb = cp.tile([128, 128], f32)
            q = cp.tile([128, NCHUNKS + 1], f32)

            # DMA plan.  V/A are host-replicated at the 4 PE row-group
            # partition bases.  Critical pieces (A slots 0-1, V block-0
            # right half, V block 1) go full-partition on sync in column
            # pieces (the dep tracker is column-granular); the bulk (blocks
            # 2-7) is read per row-group straight from the replicated blob
            # (4 x 60KB on gpsimd).  V cols [0:512) are used by nothing
            # (block 0 only has its trimmed diagonal chunk).  The Scalar
            # engine issues no DMAs so its ACT_TABLE_LOAD runs at t=0.
            nc.sync.dma_start(A_sb[:, 0:256], A_d[:, 0:256])
            nc.sync.dma_start(V_sb[:, 512:1024], V_d[:, 512:1024])
            nc.sync.dma_start(V_sb[:, 1024:2048], V_d[:, 1024:2048])
            nc.sync.dma_start(A_sb[:, 256:1024], A_d[:, 256:1024])
            for j in range(2, 8):
                nc.gpsimd.dma_start(V_sb[:, 1024 * j:1024 * (j + 1)],
                                    V_d[:, 1024 * j:1024 * (j + 1)])
            nc.sync.dma_start(TP_sb[:], TP_d[:])

            # Emit matmuls in rounds with interleaved halves so consecutive
            # MMs target different PE row-groups (4-way overlap).  Early
            # rounds are small so the first chunks only need the V blocks
            # that have already arrived (no PE FIFO head-of-line stalls).
            rounds = []
            pos = 0
            for size in (1, 2, 3):
                rounds.append(list(range(pos, pos + size)))
                pos += size
            while pos < NCHUNKS:
                rounds.append(list(range(pos, min(pos + 4, NCHUNKS))))
                pos += 4
            ka = 0
            kv = NACT + 1
            act_set = set(ACT_IDX)
            for ridx, grp in enumerate(rounds):
                pss = {}
                for i in grp:
                    on_act = i in act_set
                    pool = pap if on_act else pvp
                    w = 512 if i in DIAG else 1024
                    pss[i] = pool.tile([128, w], f32,
                                       tag="pa" if on_act else "pv",
                                       name=f"ps{i}")
                # DVE-destined chunks first within each wave: DVE frees its
                # tiles sooner, so the PE never stalls ahead of DVE work.
                order = sorted(grp, key=lambda i: (i in act_set, i))
                for h in range(2):
                    for i in order:
                        if h == 1 and i in DIAG:
                            continue
                        s, b = CHUNKS[i]
                        g = 32 * (i % 4)
                        # diagonal chunks cover only the right half of
                        # their block
                        c0 = 1024 * b + 512 * (1 if i in DIAG else h)
                        nc.tensor.matmul(
                            pss[i][:, 512 * h:512 * (h + 1)],
                            A_sb[g:g + KDIM, 128 * s:128 * (s + 1)],
                            V_sb[g:g + KDIM, c0:c0 + 512],
                            start=True, stop=True,
                            tile_position=(g, 0),
                        )
                for i in order:
                    ps = pss[i]
                    if i in act_set:
                        nc.scalar.activation(ps[:], ps[:], Act.Relu,
                                             accum_out=q[:, ka:ka + 1])
                        ka += 1
                    else:
                        nc.vector.tensor_scalar(
                            ps[:], ps[:], 0.0, None, op0=Alu.max,
                            op1=Alu.add, accum_out=q[:, kv:kv + 1],
                        )
                        kv += 1
            assert ka == NACT and kv == NCHUNKS + 1

            # MSE, off the reducers' critical path: subtract on GpSimd (idle
            # engine; waits on the late TP DMA without blocking DVE), square
            # + accumulate on ACT at the very end.
            d_sb = cp.tile([128, 64], f32)
            nc.gpsimd.tensor_sub(d_sb[:], TP_sb[:, 64:128], TP_sb[:, 0:64])
            msq = cp.tile([128, 64], f32)
            nc.scalar.activation(msq[:], d_sb[:], Act.Square,
                                 accum_out=q[:, NACT:NACT + 1])

            nc.sync.dma_start(Q_d[:], q[:])

    nc.compile()
    return nc


def _host_inputs(pred: np.ndarray, target: np.ndarray):
    """Sort by target; build bf16 A (replicated per core) and V (replicated,
    shared); compute the exact correction: garbage lower-triangle content of
    the 8 diagonal chunks per core, plus exact-tie pairs."""
    import ml_dtypes

    order = np.argsort(target, kind="stable")
    ts32 = target[order]
    ps32 = pred[order]
    ts = ts32.astype(np.float64)
    ps = ps32.astype(np.float64)

    c = _poly_coeffs()
    V = np.empty((KDIM, B), dtype=np.float64)
    V[0] = 1.0
    for k in range(1, D + 1):
        V[k] = ts ** k
    V[KDIM - 1] = ps

    # A_k(t_i) = sum_{n >= max(k,1)} c_n * C(n,k) * (-t_i)^(n-k)
    A = np.zeros((KDIM, B), dtype=np.float64)
    for k in range(0, D + 1):
        for n in range(max(k, 1), D + 1):
            A[k] += c[n - 1] * comb(n, k) * (-ts) ** (n - k)
    A[0] += ps          # fold +p_i into the constant row
    A[KDIM - 1] = -1.0

    Vq = V.astype(ml_dtypes.bfloat16)
    Aq = A.astype(ml_dtypes.bfloat16)
    Vf = Vq.astype(np.float32)
    Af = Aq.astype(np.float32)

    VR = np.zeros((128, B), dtype=ml_dtypes.bfloat16)
    for g in range(4):
        VR[32 * g:32 * g + KDIM, :] = Vq

    TP = np.zeros((128, 128), dtype=np.float32)
    TP[:, 0:64] = ts32.reshape(128, 64)
    TP[:, 64:128] = ps32.reshape(128, 64)

    jloc = np.arange(1024)[None, :]
    prow = np.arange(128)[:, None]
    in_maps = []
    garbage = np.float64(0.0)  # summed over all cores
    for core in range(NCORES):
        rows = np.concatenate(
            [128 * (8 * s + core) + np.arange(128) for s in range(NSLOTS)])
        Acore = Aq[:, rows]                       # [KDIM, 1024]
        AR = np.zeros((128, 1024), dtype=ml_dtypes.bfloat16)
        for g in range(4):
            AR[32 * g:32 * g + KDIM, :] = Acore
        in_maps.append({"AR": AR, "VR": VR, "TP": TP})

        # Diagonal chunk (s, b=s): the device processes only jloc in
        # [512, 1024).  Net correction = included-garbage (jloc >= 512 and
        # j <= i) minus missing-real (jloc < 512 and j > i).
        Acf = Acore.astype(np.float32)
        for s in range(NSLOTS):
            m = (Acf[:, 128 * s:128 * (s + 1)].T
                 @ Vf[:, 1024 * s:1024 * (s + 1)])
            r = np.maximum(m, np.float32(0.0)).astype(np.float64)
            proc = jloc >= 512
            tri = jloc > (128 * core + prow)
            garbage += r[proc & ~tri].sum()
            garbage -= r[~proc & tri].sum()

    # exact ties (t_i == t_j in fp32, i<j sorted): device adds
    # relu(Af(:,i).Vf(:,j)); reference wants 0.  Subtract device value.
    ties = np.float64(0.0)
    uq, inv, cnt = np.unique(ts32, return_inverse=True, return_counts=True)
    for gidx in np.nonzero(cnt > 1)[0]:
        idx = np.nonzero(inv == gidx)[0]
        for a in range(len(idx)):
            for bb in range(a + 1, len(idx)):
                i, j = idx[a], idx[bb]
                mv = np.float32(Af[:, i] @ Vf[:, j])
                ties += max(float(mv), 0.0)

    return in_maps, garbage + ties


def kernel(pred: np.ndarray, target: np.ndarray):
    from concourse.bass_utils import run_bass_kernel_spmd

    pred = np.ascontiguousarray(np.asarray(pred, dtype=np.float32))
    target = np.ascontiguousarray(np.asarray(target, dtype=np.float32))
    assert pred.shape == (B,) and target.shape == (B,)

    if "nc" not in _CACHE:
        _CACHE["nc"] = _build_program()
    nc = _CACHE["nc"]

    in_maps, correction = _host_inputs(pred, target)
    res = run_bass_kernel_spmd(nc, in_maps, list(range(NCORES)))
    _CACHE["last_results"] = res

    total = np.float64(0.0)
    for core in range(NCORES):
        qq = res.results[core]["Q"].astype(np.float64)
        total += qq[:, :NACT].sum() + qq[:, NACT + 1:].sum()
    K = B * (B - 1) // 2
    rank = (total - correction) / K
    mse = res.results[0]["Q"][:, NACT].astype(np.float64).sum() / B
    combined = MSE_WEIGHT * mse + RANK_WEIGHT * rank
    return (
        np.float32(combined),
        np.float32(mse),
        np.float32(rank),
    )
